# revision 83
# baseline (speedup 1.0000x reference)
"""MLA (DeepSeek-style multi-head latent attention) forward on 8 trn2 cores.

Layout v3: hybrid-replicated KV latent + fp8 DoubleRow 3-term matmuls.

Key ideas vs v2 (the 269us baseline):
- The kv LoRA-A latent for the FRONT 3/4 of the sequence is computed
  redundantly on every core (cheap in fp8 DoubleRow); only the BACK 1/4 is
  sequence-sharded and AllGathered. This shrinks the AllGather from 74us to
  ~30us of exclusive collective-device time and lets it start at ~8us, so
  the q AllToAlls (still split per head for head-0 pipelining) finish ~40us
  earlier.
- All contraction>=256 matmuls (q_a, q_b, kv_a, o_proj) run as 3-term
  hi/lo fp8e4m3 residual products in DoubleRow mode (0.5 cycles/row, two
  128-rows of contraction per instruction): hi*hi + hi*lo + lo*hi at 0.75x
  the bf16 PE cost and ~bf16 accuracy. Weights are pre-split on the host
  with a x32 power-of-2 scale folded into the rms epsilon / softmax scale /
  output-copy scale, so no extra device ops for scaling.
- Attention (scores / AV / softmax denominator) stays bf16: DoubleRow can't
  win at contraction 192/128 without giving up one operand's hi/lo split.
"""
import numpy as np
import ml_dtypes

import concourse.bass as bass
import concourse.tile as tile
from concourse import bacc, mybir
from concourse.bass_utils import run_bass_kernel_spmd

F32 = mybir.dt.float32
BF16 = mybir.dt.bfloat16
FP8 = mybir.dt.float8e4
NPBF = ml_dtypes.bfloat16
NPF8 = ml_dtypes.float8_e4m3
DR = mybir.MatmulPerfMode.DoubleRow

HID = 2048
S = 2048
H = 16
QL = 1536
KVL = 512
NOPE = 128
RP = 64
VD = 128
QD = NOPE + RP              # 192
SCALE = QD ** -0.5
EPS = 1e-6
ROPE_THETA = 10000.0

NC = 8
HPC = 2                     # heads per core
SSH = S // NC               # 256-seq shard (q path)
FRONT = 1536                # replicated kv-latent prefix
BACK = S - FRONT            # gathered suffix
BK = BACK // NC             # 64-per-core back shard
KT = HID // 128             # 16
QLT = QL // 128             # 12
CT = KVL // 128             # 4
SB = 512                    # attention query block
NSB = S // SB               # 4
NEG = -30000.0
SW = 32.0                   # fp8 weight pre-scale
EPS_S = EPS * SW * SW       # rms eps in the x32 domain

_CACHE = {}
LAST_RESULT = None


def _build_program():
    nc = bacc.Bacc("TRN2", target_bir_lowering=False, debug=False,
                   num_devices=NC)
    t = lambda name, shape, dt=FP8: nc.dram_tensor(
        name, shape, dt, kind="ExternalInput").ap()
    d = {
        "xsh_h": t("xsh_h", [128, KT, SSH]),
        "xsh_l": t("xsh_l", [128, KT, SSH]),
        "xf_h": t("xf_h", [128, KT, FRONT + BK]),
        "xf_l": t("xf_l", [128, KT, FRONT + BK]),
        "wqa_h": t("wqa_h", [128, KT, QL]),
        "wqa_l": t("wqa_l", [128, KT, QL]),
        "wkva_h": t("wkva_h", [128, KT, KVL + RP]),
        "wkva_l": t("wkva_l", [128, KT, KVL + RP]),
        "wqb_h": t("wqb_h", [128, QLT, H * QD]),
        "wqb_l": t("wqb_l", [128, QLT, H * QD]),
        "wo_h": t("wo_h", [128, HPC, HID]),
        "wo_l": t("wo_l", [128, HPC, HID]),
        "wk": t("wk16", [128, CT, HPC * NOPE], BF16),
        "wv": t("wv16", [128, CT, HPC * VD], BF16),
        "cosq": t("cosq", [128, SSH], BF16),
        "sinq": t("sinq", [128, SSH], BF16),
        "cosk": t("cosk", [RP, S], BF16),
        "sink": t("sink", [RP, S], BF16),
        "msk": t("maskadd", [128, 4, SB], F32),
        "rotq": t("rotq16", [128, 128], BF16),
    }
    d_out = nc.dram_tensor("out", [S, HID], BF16, kind="ExternalOutput").ap()

    with tile.TileContext(nc) as tc:
        _mla(tc, d, d_out)
    nc.compile()
    return nc


def _mm3(nc, psum, lh, ll, rh, rl, np_, first, last):
    """3-term hi/lo fp8 DoubleRow accumulation into psum.

    lh/ll, rh/rl: callables pair-index -> AP slice [*, 2, *].
    np_: number of contraction pairs (contraction = 256*np_).
    """
    k = 0
    for p in range(np_):
        for a, b in ((lh, rh), (lh, rl), (ll, rh)):
            nc.tensor.matmul(psum, a(p), b(p), start=(first and k == 0),
                             stop=(last and k == 3 * np_ - 1), perf_mode=DR)
            k += 1


def _mla(tc, d, d_out):
    nc = tc.nc
    Exp = mybir.ActivationFunctionType.Exp
    Sqrt = mybir.ActivationFunctionType.Sqrt
    Copy = mybir.ActivationFunctionType.Copy
    Mul = mybir.AluOpType.mult
    Sub = mybir.AluOpType.subtract
    groups = [list(range(NC))]

    with nc.allow_low_precision(reason="fp8/bf16 pipeline"), \
         tc.tile_pool(name="pdram", bufs=1, space="DRAM") as pdram, \
         tc.tile_pool(name="pconst", bufs=1) as pc, \
         tc.tile_pool(name="pglob", bufs=1) as pg:
        # ---- DRAM bounce buffers for collectives ----
        ag_in = pdram.tile([KVL + RP, BK], BF16)
        ag_out = pdram.tile([NC, KVL + RP, BK], BF16)
        aa_in = [pdram.tile([NC, QD, SSH], BF16, name=f"aain{i}")
                 for i in range(HPC)]
        aa_out = [pdram.tile([NC, QD, SSH], BF16, name=f"aaout{i}")
                  for i in range(HPC)]

        # ---- small constants ----
        ones_c = pc.tile([128, 1], BF16)
        nc.vector.memset(ones_c, 1.0)
        ones_r = pc.tile([1, 128], BF16)
        nc.vector.memset(ones_r, 1.0)
        ones8 = pc.tile([128, 2, 64], FP8)
        nc.vector.memset(ones8, 1.0)
        ebias = pc.tile([128, 1], F32)
        nc.vector.memset(ebias, -1.2)
        eps1 = pc.tile([1, 1], F32)
        nc.vector.memset(eps1, EPS_S)
        warm = pc.tile([1, 2], F32, name="actwarm")
        nc.scalar.activation(warm[0:1, 0:1], eps1[:], Sqrt)
        nc.scalar.activation(warm[0:1, 1:2], eps1[:], Exp)
        rotq = pc.tile([128, 128], BF16)
        nc.sync.dma_start(out=rotq, in_=d["rotq"])
        cosq = pc.tile([128, SSH], BF16)
        nc.sync.dma_start(out=cosq, in_=d["cosq"])
        sinq = pc.tile([128, SSH], BF16)
        nc.sync.dma_start(out=sinq, in_=d["sinq"])
        # global (stage A+F) tensors
        wkva_h = pg.tile([128, KT, KVL + RP], FP8)
        wkva_l = pg.tile([128, KT, KVL + RP], FP8)

        # =============== stage A: q path + back-shard kv_a ===============
        with tc.tile_pool(name="pwA", bufs=1) as pw, \
             tc.tile_pool(name="pA", bufs=1) as pa, \
             tc.tile_pool(name="pAs", bufs=2) as pas, \
             tc.tile_pool(name="ppA", bufs=3, space="PSUM") as ppa, \
             tc.tile_pool(name="ppSt", bufs=2, space="PSUM") as ppst, \
             tc.tile_pool(name="ppM", bufs=2, space="PSUM") as ppm:
            xsh_h = pw.tile([128, KT, SSH], FP8)
            xsh_l = pw.tile([128, KT, SSH], FP8)
            # back-shard x: slice of xf at columns FRONT..FRONT+BK
            xb_h = pw.tile([128, KT, BK], FP8)
            xb_l = pw.tile([128, KT, BK], FP8)
            # ordered load sequence on the SP queue: the DMA-engine pool is an
            # exclusive FIFO, so issue order IS service order. Back-shard
            # inputs first (gates the AllGather), then the q-path weights in
            # consumption-order column chunks so compute streams behind them.
            nc.sync.dma_start(out=xb_h, in_=d["xf_h"][:, :, FRONT:])
            nc.sync.dma_start(out=xb_l, in_=d["xf_l"][:, :, FRONT:])
            nc.sync.dma_start(out=wkva_h, in_=d["wkva_h"])
            nc.sync.dma_start(out=wkva_l, in_=d["wkva_l"])
            wqa_h = pw.tile([128, KT, QL], FP8)
            wqa_l = pw.tile([128, KT, QL], FP8)
            wqb_h = pw.tile([128, QLT, H * QD], FP8)
            wqb_l = pw.tile([128, QLT, H * QD], FP8)

            # --- back-shard kv LoRA-A (3-term DR) + rms ---
            bkvu = pa.tile([128, CT, BK], BF16, name="bkvu")
            sqb = pa.tile([128, CT, BK], BF16, name="sqb")
            kpb = pa.tile([RP, BK], BF16, name="kpb")
            p_bst = ppst.tile([1, BK], F32, tag="st", name="bstat")
            for m in range(CT + 1):
                mw = 128 if m < CT else RP
                p_a = ppa.tile([128, BK], F32, tag="a")
                _mm3(nc, p_a[:mw, :],
                     lambda p: wkva_h[:, 2 * p:2 * p + 2, m * 128:m * 128 + mw],
                     lambda p: wkva_l[:, 2 * p:2 * p + 2, m * 128:m * 128 + mw],
                     lambda p: xb_h[:, 2 * p:2 * p + 2, :],
                     lambda p: xb_l[:, 2 * p:2 * p + 2, :],
                     KT // 2, True, True)
                if m < CT:
                    nc.vector.tensor_copy(bkvu[:, m, :], p_a[:])
                    nc.vector.tensor_tensor(sqb[:, m, :], p_a[:], bkvu[:, m, :],
                                            Mul)
                else:
                    # raw kpe for own back shard -> staging rows KVL..
                    nc.vector.tensor_copy(kpb[:], p_a[:mw, :])
            # q-path loads gated on the first back-shard output so their FIFO
            # slots come after the back-shard inputs but alongside the rms
            # chain; the collective staging only queues ~6us behind them.
            gates = pc.tile([1, 32], BF16, name="gates")
            gate_state = {"tok": bkvu[0:1, 0, 0:1], "i": 0}

            def gated_dma(dst_tile, region, src):
                nc.sync.dma_start(out=dst_tile, in_=src)

            gated_dma(xsh_h, xsh_h[0:1, 0, 0:1], d["xsh_h"])
            gated_dma(xsh_l, xsh_l[0:1, 0, 0:1], d["xsh_l"])
            for m in range(CT):
                nc.tensor.matmul(p_bst[:], ones_c[:], sqb[:, m, :],
                                 start=(m == 0), stop=(m == CT - 1))
            rms_b = pas.tile([1, BK], BF16, tag="r1")
            nc.scalar.activation(rms_b[:], p_bst[:], Sqrt, scale=1.0 / KVL,
                                 bias=eps1[:])
            p_bb = ppm.tile([128, BK], F32, tag="m")
            nc.tensor.matmul(p_bb[:], ones_r[:], rms_b[:], start=True, stop=True)
            invb = pas.tile([128, BK], BF16, tag="r2")
            nc.vector.reciprocal(invb[:], p_bb[:])
            ckvb = pa.tile([128, CT, BK], BF16, name="ckvb")
            for m in range(CT):
                nc.vector.tensor_tensor(ckvb[:, m, :], bkvu[:, m, :], invb[:], Mul)
            nc.gpsimd.dma_start(
                out=ag_in[0:KVL, :].rearrange("(t p) c -> p t c", p=128),
                in_=ckvb[:])
            nc.gpsimd.dma_start(out=ag_in[KVL:, :], in_=kpb[:])
            # --- collective #1: AllGather back-shard latent+kpe ---
            nc.gpsimd.collective_compute(
                "AllGather", mybir.AluOpType.bypass, replica_groups=groups,
                ins=[ag_in[:].opt()], outs=[ag_out[:].opt()])
            # The tile scheduler orders by data deps, so dep-free bulk loads
            # would enter the shared DMA-engine FIFO ahead of the collective
            # staging. Gate each bulk chunk behind the previous one via a
            # write-after-read chain rooted at agmark (a tiny read of ag_in
            # that lands right after the staging writes).
            # agmark lands right after the AllGather staging is written; an
            # Activation-engine gate (activation reading the DMA's dest cell
            # with agmark as bias) makes each bulk load's FIFO slot follow
            # the staging without polluting the DVE SEQ.
            def act_gated_dma(dst_tile, region, src):
                # Pool-SEQ-timed gate: the gpsimd sequencer is blocked on the
                # AllGather's staging sems until the staging transfers land,
                # so this tiny read of the DMA's dest cell (WAR) executes at
                # staging-complete and admits the bulk load to the DMA FIFO
                # only after the collective staging — no token needed.
                g = gates[0:1, gate_state["i"] % 32:gate_state["i"] % 32 + 1]
                nc.gpsimd.tensor_tensor(g, region, ones_r[0:1, 0:1], Mul)
                nc.sync.dma_start(out=dst_tile, in_=src)
                gate_state["i"] += 1

            gate_state["fn"] = act_gated_dma
            for cc in range(3):
                cs = slice(cc * 512, (cc + 1) * 512)
                act_gated_dma(wqa_h[:, :, cs],
                              wqa_h[0:1, 0, cc * 512:cc * 512 + 1],
                              d["wqa_h"][:, :, cs])
                act_gated_dma(wqa_l[:, :, cs],
                              wqa_l[0:1, 0, cc * 512:cc * 512 + 1],
                              d["wqa_l"][:, :, cs])
            for cc in range(3):
                cs = slice(cc * 1024, (cc + 1) * 1024)
                act_gated_dma(wqb_h[:, :, cs],
                              wqb_h[0:1, 0, cc * 1024:cc * 1024 + 1],
                              d["wqb_h"][:, :, cs])
                act_gated_dma(wqb_l[:, :, cs],
                              wqb_l[0:1, 0, cc * 1024:cc * 1024 + 1],
                              d["wqb_l"][:, :, cs])

            # --- q LoRA-A (3-term DR) ---
            qlu_h = pa.tile([128, QLT, SSH], FP8, name="qluh")
            qlu_l = pa.tile([128, QLT, SSH], FP8, name="qlul")
            p_qst = ppst.tile([1, SSH], F32, tag="st", name="qstat")
            sqq = pa.tile([128, QLT, SSH], BF16, name="sqq")
            for k in range(QLT):
                p_a = ppa.tile([128, SSH], F32, tag="a")
                _mm3(nc, p_a[:],
                     lambda p: wqa_h[:, 2 * p:2 * p + 2, k * 128:(k + 1) * 128],
                     lambda p: wqa_l[:, 2 * p:2 * p + 2, k * 128:(k + 1) * 128],
                     lambda p: xsh_h[:, 2 * p:2 * p + 2, :],
                     lambda p: xsh_l[:, 2 * p:2 * p + 2, :],
                     KT // 2, True, True)
                nc.scalar.activation(qlu_h[:, k, :], p_a[:], Copy)
                nc.vector.tensor_tensor(qlu_l[:, k, :], p_a[:], qlu_h[:, k, :], Sub)
                nc.vector.tensor_tensor(sqq[:, k, :], p_a[:], qlu_h[:, k, :], Mul)
            for k in range(QLT):
                nc.tensor.matmul(p_qst[:], ones_c[:], sqq[:, k, :],
                                 start=(k == 0), stop=(k == QLT - 1))
            rms_q = pas.tile([1, SSH], BF16, tag="r1")
            nc.scalar.activation(rms_q[:], p_qst[:], Sqrt, scale=1.0 / QL,
                                 bias=eps1[:])
            p_bq = ppm.tile([128, SSH], F32, tag="m")
            nc.tensor.matmul(p_bq[:], ones_r[:], rms_q[:], start=True, stop=True)
            invq = pas.tile([128, SSH], F32, tag="r2", name="invq")
            nc.vector.reciprocal(invq[:], p_bq[:])

            # --- q_b (3-term DR): rope tiles first, then parity order ---
            q16 = pa.tile([128, H + NC, SSH], BF16, name="q16")

            def qb_group(mt):
                p_q = ppa.tile([128, SSH], F32, tag="a")
                _mm3(nc, p_q[:],
                     lambda p: wqb_h[:, 2 * p:2 * p + 2, mt * 128:(mt + 1) * 128],
                     lambda p: wqb_l[:, 2 * p:2 * p + 2, mt * 128:(mt + 1) * 128],
                     lambda p: qlu_h[:, 2 * p:2 * p + 2, :],
                     lambda p: qlu_l[:, 2 * p:2 * p + 2, :],
                     QLT // 2, True, True)
                nc.vector.tensor_tensor(q16[:, mt, :], p_q[:], invq[:], Mul)

            for mt in range(NC):
                qb_group(mt)
            for dd in range(NC):
                p_rq = ppm.tile([128, SSH], F32, tag="m")
                nc.tensor.matmul(p_rq[:], rotq[:], q16[:, dd, :],
                                 start=True, stop=True)
                rq16 = pas.tile([128, SSH], BF16, tag="rk", name="rq16")
                nc.vector.tensor_copy(rq16[:], p_rq[:])
                t1q = pas.tile([128, SSH], BF16, tag="t1")
                nc.vector.tensor_tensor(t1q[:], q16[:, dd, :], cosq[:], Mul)
                t2q = pas.tile([128, SSH], BF16, tag="t2")
                nc.vector.tensor_tensor(t2q[:], rq16[:], sinq[:], Mul)
                nc.vector.tensor_tensor(q16[:, dd, :], t1q[:], t2q[:],
                                        mybir.AluOpType.add)
            for mt in range(NC, NC + 8):
                qb_group(mt)
            nc.scalar.dma_start(
                out=aa_in[0][:, 0:NOPE, :].rearrange("j p c -> p j c"),
                in_=q16[:, 8:16, :])
            nc.scalar.dma_start(
                out=aa_in[0][:, NOPE:QD, :].rearrange("j p c -> p j c"),
                in_=q16[0:RP, 0:NC, :])
            nc.gpsimd.collective_compute(
                "AllToAll", mybir.AluOpType.bypass, replica_groups=groups,
                ins=[aa_in[0][:].opt()], outs=[aa_out[0][:].opt()])
            for mt in range(NC + 8, NC + 16):
                qb_group(mt)
            nc.scalar.dma_start(
                out=aa_in[1][:, 0:NOPE, :].rearrange("j p c -> p j c"),
                in_=q16[:, 16:24, :])
            nc.scalar.dma_start(
                out=aa_in[1][:, NOPE:QD, :].rearrange("j p c -> p j c"),
                in_=q16[RP:128, 0:NC, :])
            nc.gpsimd.collective_compute(
                "AllToAll", mybir.AluOpType.bypass, replica_groups=groups,
                ins=[aa_in[1][:].opt()], outs=[aa_out[1][:].opt()])

        # =============== stage F: front kv_a (replicated, 3-term DR) =========
        with tc.tile_pool(name="pg2", bufs=1) as pg2:
            ckv = pg2.tile([128, CT, S], BF16)          # assembled latent
            kpr = pg2.tile([RP, S], BF16, name="kpraw")  # raw kpe (x32)
            kv = {
                "wk": pg2.tile([128, CT, HPC * NOPE], BF16, name="wk"),
                "wv": pg2.tile([128, CT, HPC * VD], BF16, name="wv"),
                "kn": [pg2.tile([128, S], BF16, name=f"kn{h}")
                       for h in range(HPC)],
                "vst_h": pg2.tile([128, S // 128, HPC * VD], FP8, name="vsth"),
                "vst_l": pg2.tile([128, S // 128, HPC * VD], FP8, name="vstl"),
            }
            _stage_f(tc, d, ckv, kpr, wkva_h, wkva_l, ones_c, ones_r, eps1,
                     gates, gate_state, kv)
            _stage_b(tc, d, d_out, ckv, kpr, ag_out, aa_in, aa_out, ones_r,
                     ones8, ebias, rotq, gates, gate_state, pg2, kv)


def _kvb_cb(nc, ckv, kv, cb):
    """kv_b expansion for one 512-key block: kn both heads + v hi/lo."""
    tc_pool = kv["ppc"]
    Copy = mybir.ActivationFunctionType.Copy
    Sub = mybir.AluOpType.subtract
    cs = slice(cb * 512, (cb + 1) * 512)
    for h in range(HPC):
        p_k = tc_pool.tile([128, 512], mybir.dt.float32, tag="c")
        for t in range(CT):
            nc.tensor.matmul(p_k[:], kv["wk"][:, t, h * NOPE:(h + 1) * NOPE],
                             ckv[:, t, cs], start=(t == 0), stop=(t == CT - 1))
        nc.any.tensor_copy(kv["kn"][h][:, cs], p_k[:])
    for sb in range(cb * 4, cb * 4 + 4):
        p_v = tc_pool.tile([128, HPC * VD], mybir.dt.float32, tag="c")
        for t in range(CT):
            nc.tensor.matmul(p_v[:], ckv[:, t, sb * 128:(sb + 1) * 128],
                             kv["wv"][:, t, :], start=(t == 0),
                             stop=(t == CT - 1))
        nc.any.tensor_copy(kv["vst_h"][:, sb, :], p_v[:])
        nc.vector.tensor_tensor(kv["vst_l"][:, sb, :], p_v[:],
                                kv["vst_h"][:, sb, :], Sub)


def _stage_f(tc, d, ckv, kpr, wkva_h, wkva_l, ones_c, ones_r, eps1, gates,
             gate_state, kv):
    nc = tc.nc
    Sqrt = mybir.ActivationFunctionType.Sqrt
    Mul = mybir.AluOpType.mult

    if True:
        with tc.tile_pool(name="pwF", bufs=1) as pwf, \
             tc.tile_pool(name="pF", bufs=1) as pf, \
             tc.tile_pool(name="pFs", bufs=4) as pfs, \
             tc.tile_pool(name="ppF", bufs=3, space="PSUM") as ppf, \
             tc.tile_pool(name="ppFSt", bufs=2, space="PSUM") as ppfst, \
             tc.tile_pool(name="ppFM", bufs=2, space="PSUM") as ppfm:
            xf_h = pwf.tile([128, KT, FRONT], FP8)
            xf_l = pwf.tile([128, KT, FRONT], FP8)
            def gated_dma(dst_tile, region, src):
                gate_state["fn"](dst_tile, region, src)

            for q3 in range(FRONT // 512):
                cs = slice(q3 * 512, (q3 + 1) * 512)
                gated_dma(xf_h[:, :, cs], xf_h[0:1, 0, q3 * 512:q3 * 512 + 1],
                          d["xf_h"][:, :, cs])
                gated_dma(xf_l[:, :, cs], xf_l[0:1, 0, q3 * 512:q3 * 512 + 1],
                          d["xf_l"][:, :, cs])
            NCB = FRONT // 512                     # 3 col blocks
            fkvu = pf.tile([128, CT, FRONT], BF16, name="fkvu")
            sqf = pf.tile([128, CT, FRONT], BF16, name="sqf")
            for cb in range(NCB):
                cs = slice(cb * 512, (cb + 1) * 512)
                for m in range(CT + 1):
                    mw = 128 if m < CT else RP
                    p_f = ppf.tile([128, 512], F32, tag="f")
                    _mm3(nc, p_f[:mw, :],
                         lambda p: wkva_h[:, 2 * p:2 * p + 2, m * 128:m * 128 + mw],
                         lambda p: wkva_l[:, 2 * p:2 * p + 2, m * 128:m * 128 + mw],
                         lambda p: xf_h[:, 2 * p:2 * p + 2, cs],
                         lambda p: xf_l[:, 2 * p:2 * p + 2, cs],
                         KT // 2, True, True)
                    if m < CT:
                        nc.any.tensor_copy(fkvu[:, m, cs], p_f[:])
                        nc.any.tensor_tensor(sqf[:, m, cs], p_f[:],
                                             fkvu[:, m, cs], Mul)
                    else:
                        nc.any.tensor_copy(kpr[:, cb * 512:(cb + 1) * 512],
                                           p_f[:mw, :])
                p_fst = ppfst.tile([1, 512], F32, tag="fst")
                for m in range(CT):
                    nc.tensor.matmul(p_fst[:], ones_c[:], sqf[:, m, cs],
                                     start=(m == 0), stop=(m == CT - 1))
                rms_f = pfs.tile([1, 512], BF16, tag="fr1")
                nc.scalar.activation(rms_f[:], p_fst[:], Sqrt, scale=1.0 / KVL,
                                     bias=eps1[:])
                p_fb = ppfm.tile([128, 512], F32, tag="fm")
                nc.tensor.matmul(p_fb[:], ones_r[:], rms_f[:], start=True,
                                 stop=True)
                invf = pfs.tile([128, 512], BF16, tag="fr2")
                nc.vector.reciprocal(invf[:], p_fb[:])
                for m in range(CT):
                    nc.any.tensor_tensor(ckv[:, m, cs], fkvu[:, m, cs],
                                         invf[:], Mul)

def _stage_b(tc, d, d_out, ckv, kpr, ag_out, aa_in, aa_out, ones_r, ones8,
             ebias, rotq, gates, gate_state, pg2, kv):
    nc = tc.nc
    Exp = mybir.ActivationFunctionType.Exp
    Copy = mybir.ActivationFunctionType.Copy
    Mul = mybir.AluOpType.mult
    Sub = mybir.AluOpType.subtract

    if True:
        # =============== stage B: assemble + head-local attention ============
        with tc.tile_pool(name="pB", bufs=1) as pb, \
             tc.tile_pool(name="pBe", bufs=13) as pbe, \
             tc.tile_pool(name="pBo", bufs=4) as pbo, \
             tc.tile_pool(name="pBn", bufs=4) as pbn, \
             tc.tile_pool(name="ppS", bufs=4, space="PSUM") as pps, \
             tc.tile_pool(name="ppO", bufs=1, space="PSUM") as ppo, \
             tc.tile_pool(name="ppD", bufs=1, space="PSUM") as ppd, \
             tc.tile_pool(name="ppC", bufs=2, space="PSUM") as ppc:
            kn, vst_h, vst_l = kv["kn"], kv["vst_h"], kv["vst_l"]
            wo_h = pg2.tile([128, HPC, HID], FP8)
            wo_l = pg2.tile([128, HPC, HID], FP8)
            msk = pg2.tile([128, 4, SB], F32)
            cosk = pg2.tile([RP, S], BF16)
            sink = pg2.tile([RP, S], BF16)
            def gated_dma(dst_tile, region, src):
                gate_state["fn"](dst_tile, region, src)

            gated_dma(kv["wk"], kv["wk"][0:1, 0, 0:1], d["wk"])
            gated_dma(kv["wv"], kv["wv"][0:1, 0, 0:1], d["wv"])
            gated_dma(cosk, cosk[0:1, 0:1], d["cosk"])
            gated_dma(sink, sink[0:1, 0:1], d["sink"])
            gated_dma(wo_h, wo_h[0:1, 0, 0:1], d["wo_h"])
            gated_dma(wo_l, wo_l[0:1, 0, 0:1], d["wo_l"])
            gated_dma(msk, msk[0:1, 0, 0:1], d["msk"])
            # unpack AG: back latent + kpe into global tiles
            for t in range(CT):
                nc.gpsimd.dma_start(
                    out=ckv[:, t, FRONT:].rearrange("p (j c) -> p j c", j=NC),
                    in_=ag_out[:, t * 128:(t + 1) * 128, :].rearrange(
                        "j p c -> p j c"))
            nc.gpsimd.dma_start(
                out=kpr[:, FRONT:].rearrange("p (j c) -> p j c", j=NC),
                in_=ag_out[:, KVL:, :].rearrange("j p c -> p j c"))
            # kpe rope over full seq (cosk/sink carry the 1/SW fold)
            kpdg = pg2.tile([RP, S], BF16)
            for cb in range(S // 512):
                cs = slice(cb * 512, (cb + 1) * 512)
                p_rk = ppc.tile([128, 512], F32, tag="c", name="rotk")
                nc.tensor.matmul(p_rk[:RP, :], rotq[0:RP, 0:RP], kpr[:, cs],
                                 start=True, stop=True)
                rk16 = pbn.tile([RP, 512], BF16, tag="rk")
                nc.vector.tensor_copy(rk16[:], p_rk[:RP, :])
                t1 = pbn.tile([RP, 512], BF16, tag="t1")
                nc.vector.tensor_tensor(t1[:], kpr[:, cs], cosk[:, cs], Mul)
                t2 = pbn.tile([RP, 512], BF16, tag="t2")
                nc.vector.tensor_tensor(t2[:], rk16[:], sink[:, cs], Mul)
                nc.vector.tensor_tensor(kpdg[:, cs], t1[:], t2[:],
                                        mybir.AluOpType.add)

            qt = [pg2.tile([128, S], BF16, name=f"qt{h}") for h in range(HPC)]
            qpt = [pg2.tile([RP, S], BF16, name=f"qpt{h}") for h in range(HPC)]

            def unpack_q(h):
                nc.gpsimd.dma_start(
                    out=qt[h][:].rearrange("p (j c) -> p j c", j=NC),
                    in_=aa_out[h][:, 0:NOPE, :].rearrange("j p c -> p j c"))
                nc.gpsimd.dma_start(
                    out=qpt[h][:].rearrange("p (j c) -> p j c", j=NC),
                    in_=aa_out[h][:, NOPE:QD, :].rearrange("j p c -> p j c"))

            # --- kv_b for the gathered back block (front blocks were
            # interleaved into stage F) ---
            kv["ppc"] = ppc
            for cb in range(2):
                _kvb_cb(nc, ckv, kv, cb)

            # --- attention: heads outer, software-pipelined (as v2) ---
            ao_h = pg2.tile([128, NSB, HPC, SB], FP8, name="aoh")
            ao_l = pg2.tile([128, NSB, HPC, SB], FP8, name="aol")
            pending = None

            def finisher(fin):
                h, qb, p_o, p_d = fin
                den = pbn.tile([1, SB], BF16, tag="den")
                nc.vector.tensor_copy(den[:], p_d[0:1, :])
                p_b = ppc.tile([128, SB], F32, tag="c", name="bcast")
                nc.tensor.matmul(p_b[:], ones_r[:], den[:], start=True, stop=True)
                rec = pbn.tile([128, SB], F32, tag="rec")
                nc.vector.reciprocal(rec[:], p_b[:])
                aot = pbn.tile([128, SB], BF16, tag="aot")
                nc.vector.tensor_tensor(aot[:], p_o[:], rec[:], Mul)
                nc.any.tensor_copy(ao_h[:, qb, h, :], aot[:])
                nc.any.tensor_tensor(ao_l[:, qb, h, :], aot[:],
                                     ao_h[:, qb, h, :], Sub)

            def oproj(qb):
                for st in range(SB // 128):
                    sc = slice(qb * SB + st * 128, qb * SB + (st + 1) * 128)
                    ot = pbo.tile([128, HID], BF16, tag="ot")
                    for nb in range(HID // SB):
                        ncols = bass.ts(nb, SB)
                        p_c = ppc.tile([128, SB], F32, tag="c")
                        aoh_ = ao_h[:, qb, :, st * 128:(st + 1) * 128]
                        aol_ = ao_l[:, qb, :, st * 128:(st + 1) * 128]
                        nc.tensor.matmul(p_c[:], aoh_, wo_h[:, :, ncols],
                                         start=True, stop=False, perf_mode=DR)
                        nc.tensor.matmul(p_c[:], aoh_, wo_l[:, :, ncols],
                                         start=False, stop=False, perf_mode=DR)
                        nc.tensor.matmul(p_c[:], aol_, wo_h[:, :, ncols],
                                         start=False, stop=True, perf_mode=DR)
                        nc.vector.tensor_scalar_mul(ot[:, ncols], p_c[:],
                                                    1.0 / SW)
                    nc.sync.dma_start(out=d_out[sc, :], in_=ot[:])

            for h in range(HPC):
                unpack_q(h)
                for qb in range(NSB):
                    qcols = bass.ts(qb, SB)
                    nk = 4 * (qb + 1)
                    npair = nk // 2
                    p_o = ppo.tile([128, SB], F32, tag="o")
                    p_d = ppd.tile([64, SB], F32, tag="d")
                    ework = []

                    def av_den(pp, e2_):
                        vs = slice(2 * pp, 2 * pp + 2)
                        hv = slice(h * VD, (h + 1) * VD)
                        nc.tensor.matmul(p_o[:], vst_h[:, vs, hv], e2_[:],
                                         start=(pp == 0), stop=False,
                                         perf_mode=DR)
                        nc.tensor.matmul(p_o[:], vst_l[:, vs, hv], e2_[:],
                                         start=False, stop=(pp == npair - 1),
                                         perf_mode=DR)
                        nc.tensor.matmul(p_d[:], ones8[:], e2_[:],
                                         start=(pp == 0), stop=(pp == npair - 1),
                                         perf_mode=DR)

                    for pp in range(npair):
                        e2 = pbe.tile([128, 2, SB], FP8, tag="e")
                        for j in range(2):
                            ik = 2 * pp + j
                            kc = slice(ik * 128, (ik + 1) * 128)
                            p_s = pps.tile([128, SB], F32, tag="s")
                            nc.tensor.matmul(p_s[:], kn[h][:, kc], qt[h][:, qcols],
                                             start=True, stop=False)
                            nc.tensor.matmul(p_s[:], kpdg[:, kc], qpt[h][:, qcols],
                                             start=False, stop=True)
                            if ik == 3 and pending is not None:
                                fin, oqb = pending
                                finisher(fin)
                                pending = None
                                if oqb is not None:
                                    oproj(oqb)
                            r = ik - 4 * qb
                            if r >= 0:
                                nc.vector.tensor_tensor(p_s[:], p_s[:],
                                                        msk[:, r, :],
                                                        mybir.AluOpType.add)
                            nc.scalar.activation(e2[:, j, :], p_s[:], Exp,
                                                 scale=SCALE / SW, bias=ebias[:])
                        ework.append((pp, e2))
                        if len(ework) == 3:
                            av_den(*ework.pop(0))
                    for item in ework:
                        av_den(*item)
                    # deferred kv_b for back key blocks: emitted after h0's
                    # early query blocks so head-0 attention reaches the PE
                    # stream as soon as AllToAll-1 lands.
                    if h == 0 and qb < 2:
                        _kvb_cb(nc, ckv, kv, 2 + qb)
                    pending = ((h, qb, p_o, p_d),
                               qb if h == HPC - 1 else None)
            fin, oqb = pending
            finisher(fin)
            if oqb is not None:
                oproj(oqb)


def _host_constants():
    inv_freq = 1.0 / (ROPE_THETA ** (np.arange(0, RP, 2, dtype=np.float32) / RP))
    t = np.arange(S, dtype=np.float32)
    freqs = np.outer(t, inv_freq)
    emb = np.concatenate([freqs, freqs], -1)          # [S, 64]
    cos, sin = np.cos(emb), np.sin(emb)
    cosq = np.concatenate([cos.T, cos.T], 0).astype(np.float32)   # [128, S]
    sinq = np.concatenate([sin.T, sin.T], 0).astype(np.float32)
    cosk = (cos.T / SW).astype(np.float32)            # [64, S], 1/SW folded
    sink = (sin.T / SW).astype(np.float32)

    mska = np.zeros((128, 4, SB), np.float32)
    for r in range(4):
        for p in range(128):
            mska[p, r, :p + 128 * r] = NEG
    Q = np.zeros((RP, RP), np.float32)
    for i in range(RP // 2):
        Q[i, i + RP // 2] = -1.0
        Q[i + RP // 2, i] = 1.0
    P = np.zeros((128, 128), np.float32)
    P[:RP, :RP] = Q
    P[RP:, RP:] = Q
    rotq = P.T.copy()
    return cosq, sinq, cosk, sink, mska, rotq


def _tile3(w, kt):
    """[kt*128, F] -> [128, kt, F]"""
    return np.ascontiguousarray(
        w.reshape(kt, 128, w.shape[1]).transpose(1, 0, 2))


def _split8(w):
    """scaled hi/lo fp8 split (already-scaled input)."""
    hi = w.astype(NPF8)
    lo = (w - hi.astype(np.float32)).astype(NPF8)
    return hi, lo


def kernel(hidden_states, w_q_a, q_a_weight, w_q_b, w_kv_a, kv_a_weight,
           w_kv_b, w_o):
    global LAST_RESULT
    if "nc" not in _CACHE:
        _CACHE["nc"] = _build_program()
    nc = _CACHE["nc"]

    x = np.asarray(hidden_states, np.float32)[0]       # [S, 2048]
    xt = np.ascontiguousarray(x.T)                     # [2048, S]
    wqa_t = np.asarray(w_q_a, np.float32).T * SW       # [HID, QL] x32
    wkva_t = np.asarray(w_kv_a, np.float32).T * SW     # [HID, 576] x32
    wqb_eff = np.asarray(w_q_b, np.float32) * np.asarray(q_a_weight, np.float32)[None, :]
    wkvb_eff = np.asarray(w_kv_b, np.float32) * np.asarray(kv_a_weight, np.float32)[None, :]
    won = np.asarray(w_o, np.float32)                  # [HID, H*VD]

    # q_b output feature permutation: cols [0:1024] rope packed 2-heads/tile,
    # [1024:2048] even heads' nope, [2048:3072] odd heads' nope — matching the
    # device-side consumption order so wqb column-chunk loads stream.
    perm = np.zeros(H * QD, np.int64)
    for dd in range(NC):
        for j in range(HPC):
            hh = 2 * dd + j
            perm[dd * 128 + j * RP: dd * 128 + (j + 1) * RP] = \
                hh * QD + NOPE + np.arange(RP)
    for ei in range(8):
        perm[1024 + ei * 128: 1024 + (ei + 1) * 128] = (2 * ei) * QD + np.arange(NOPE)
    for oi in range(8):
        perm[2048 + oi * 128: 2048 + (oi + 1) * 128] = (2 * oi + 1) * QD + np.arange(NOPE)
    wqb_p = np.ascontiguousarray(wqb_eff[perm, :].T) * SW   # [QL, 3072] x32

    cosq, sinq, cosk, sink, mska, rotq = _host_constants()

    xt_h, xt_l = _split8(xt)                           # full [2048, S]
    wqa_h, wqa_l = _split8(wqa_t)
    wkva_h, wkva_l = _split8(wkva_t)
    wqb_h, wqb_l = _split8(wqb_p)

    shared = {
        "wqa_h": _tile3(wqa_h, KT), "wqa_l": _tile3(wqa_l, KT),
        "wkva_h": _tile3(wkva_h, KT), "wkva_l": _tile3(wkva_l, KT),
        "wqb_h": _tile3(wqb_h, QLT), "wqb_l": _tile3(wqb_l, QLT),
        "maskadd": mska, "rotq16": rotq.astype(NPBF),
        "cosk": cosk.astype(NPBF), "sink": sink.astype(NPBF),
    }

    in_maps = []
    for c in range(NC):
        h0, h1 = HPC * c, HPC * c + 1
        wk_t = np.concatenate(
            [wkvb_eff[h * (NOPE + VD):h * (NOPE + VD) + NOPE] for h in (h0, h1)],
            0).T
        wv_t = np.concatenate(
            [wkvb_eff[h * (NOPE + VD) + NOPE:(h + 1) * (NOPE + VD)] for h in (h0, h1)],
            0).T
        wo_t = np.stack(
            [np.ascontiguousarray(won[:, h * VD:(h + 1) * VD].T) for h in (h0, h1)],
            1) * SW                                     # [128, 2, HID] x32
        wo_hc, wo_lc = _split8(wo_t)
        cols = slice(c * SSH, (c + 1) * SSH)
        bcols = np.r_[0:FRONT, FRONT + c * BK:FRONT + (c + 1) * BK]
        im = dict(shared)
        im.update({
            "xsh_h": _tile3(np.ascontiguousarray(xt_h[:, cols].astype(np.float32)), KT).astype(NPF8),
            "xsh_l": _tile3(np.ascontiguousarray(xt_l[:, cols].astype(np.float32)), KT).astype(NPF8),
            "xf_h": _tile3(np.ascontiguousarray(xt_h[:, bcols].astype(np.float32)), KT).astype(NPF8),
            "xf_l": _tile3(np.ascontiguousarray(xt_l[:, bcols].astype(np.float32)), KT).astype(NPF8),
            "wk16": _tile3(wk_t, CT).astype(NPBF),
            "wv16": _tile3(wv_t, CT).astype(NPBF),
            "wo_h": np.ascontiguousarray(wo_hc),
            "wo_l": np.ascontiguousarray(wo_lc),
            "cosq": np.ascontiguousarray(cosq[:, cols]).astype(NPBF),
            "sinq": np.ascontiguousarray(sinq[:, cols]).astype(NPBF),
        })
        in_maps.append(im)

    res = run_bass_kernel_spmd(nc, in_maps, list(range(NC)))
    LAST_RESULT = res
    out = np.zeros((S, HID), np.float32)
    for c in range(NC):
        out += np.asarray(res.results[c]["out"]).astype(np.float32)
    return out.reshape(1, S, HID)


# revision 84
# speedup vs baseline: 1.0021x; 1.0021x over previous
"""MLA (DeepSeek-style multi-head latent attention) forward on 8 trn2 cores.

Layout v3: hybrid-replicated KV latent + fp8 DoubleRow 3-term matmuls.

Key ideas vs v2 (the 269us baseline):
- The kv LoRA-A latent for the FRONT 3/4 of the sequence is computed
  redundantly on every core (cheap in fp8 DoubleRow); only the BACK 1/4 is
  sequence-sharded and AllGathered. This shrinks the AllGather from 74us to
  ~30us of exclusive collective-device time and lets it start at ~8us, so
  the q AllToAlls (still split per head for head-0 pipelining) finish ~40us
  earlier.
- All contraction>=256 matmuls (q_a, q_b, kv_a, o_proj) run as 3-term
  hi/lo fp8e4m3 residual products in DoubleRow mode (0.5 cycles/row, two
  128-rows of contraction per instruction): hi*hi + hi*lo + lo*hi at 0.75x
  the bf16 PE cost and ~bf16 accuracy. Weights are pre-split on the host
  with a x32 power-of-2 scale folded into the rms epsilon / softmax scale /
  output-copy scale, so no extra device ops for scaling.
- Attention (scores / AV / softmax denominator) stays bf16: DoubleRow can't
  win at contraction 192/128 without giving up one operand's hi/lo split.
"""
import numpy as np
import ml_dtypes

import concourse.bass as bass
import concourse.tile as tile
from concourse import bacc, mybir
from concourse.bass_utils import run_bass_kernel_spmd

F32 = mybir.dt.float32
BF16 = mybir.dt.bfloat16
FP8 = mybir.dt.float8e4
NPBF = ml_dtypes.bfloat16
NPF8 = ml_dtypes.float8_e4m3
DR = mybir.MatmulPerfMode.DoubleRow

HID = 2048
S = 2048
H = 16
QL = 1536
KVL = 512
NOPE = 128
RP = 64
VD = 128
QD = NOPE + RP              # 192
SCALE = QD ** -0.5
EPS = 1e-6
ROPE_THETA = 10000.0

NC = 8
HPC = 2                     # heads per core
SSH = S // NC               # 256-seq shard (q path)
FRONT = 1536                # replicated kv-latent prefix
BACK = S - FRONT            # gathered suffix
BK = BACK // NC             # 64-per-core back shard
KT = HID // 128             # 16
QLT = QL // 128             # 12
CT = KVL // 128             # 4
SB = 512                    # attention query block
NSB = S // SB               # 4
NEG = -30000.0
SW = 32.0                   # fp8 weight pre-scale
EPS_S = EPS * SW * SW       # rms eps in the x32 domain

_CACHE = {}
LAST_RESULT = None


def _build_program():
    nc = bacc.Bacc("TRN2", target_bir_lowering=False, debug=False,
                   num_devices=NC)
    t = lambda name, shape, dt=FP8: nc.dram_tensor(
        name, shape, dt, kind="ExternalInput").ap()
    d = {
        "xsh_h": t("xsh_h", [128, KT, SSH]),
        "xsh_l": t("xsh_l", [128, KT, SSH]),
        "xf_h": t("xf_h", [128, KT, FRONT + BK]),
        "xf_l": t("xf_l", [128, KT, FRONT + BK]),
        "wqa_h": t("wqa_h", [128, KT, QL]),
        "wqa_l": t("wqa_l", [128, KT, QL]),
        "wkva_h": t("wkva_h", [128, KT, KVL + RP]),
        "wkva_l": t("wkva_l", [128, KT, KVL + RP]),
        "wqb_h": t("wqb_h", [128, QLT, H * QD]),
        "wqb_l": t("wqb_l", [128, QLT, H * QD]),
        "wo_h": t("wo_h", [128, HPC, HID]),
        "wo_l": t("wo_l", [128, HPC, HID]),
        "wk": t("wk16", [128, CT, HPC * NOPE], BF16),
        "wv": t("wv16", [128, CT, HPC * VD], BF16),
        "cosq": t("cosq", [128, SSH], BF16),
        "sinq": t("sinq", [128, SSH], BF16),
        "cosk": t("cosk", [RP, S], BF16),
        "sink": t("sink", [RP, S], BF16),
        "msk": t("maskadd", [128, 4, SB], F32),
        "rotq": t("rotq16", [128, 128], BF16),
    }
    d_out = nc.dram_tensor("out", [S, HID], BF16, kind="ExternalOutput").ap()

    with tile.TileContext(nc) as tc:
        _mla(tc, d, d_out)
    nc.compile()
    return nc


def _mm3(nc, psum, lh, ll, rh, rl, np_, first, last):
    """3-term hi/lo fp8 DoubleRow accumulation into psum.

    lh/ll, rh/rl: callables pair-index -> AP slice [*, 2, *].
    np_: number of contraction pairs (contraction = 256*np_).
    """
    k = 0
    for p in range(np_):
        for a, b in ((lh, rh), (lh, rl), (ll, rh)):
            nc.tensor.matmul(psum, a(p), b(p), start=(first and k == 0),
                             stop=(last and k == 3 * np_ - 1), perf_mode=DR)
            k += 1


def _mla(tc, d, d_out):
    nc = tc.nc
    Exp = mybir.ActivationFunctionType.Exp
    Sqrt = mybir.ActivationFunctionType.Sqrt
    Copy = mybir.ActivationFunctionType.Copy
    Mul = mybir.AluOpType.mult
    Sub = mybir.AluOpType.subtract
    groups = [list(range(NC))]

    with nc.allow_low_precision(reason="fp8/bf16 pipeline"), \
         tc.tile_pool(name="pdram", bufs=1, space="DRAM") as pdram, \
         tc.tile_pool(name="pconst", bufs=1) as pc, \
         tc.tile_pool(name="pglob", bufs=1) as pg:
        # ---- DRAM bounce buffers for collectives ----
        ag_in = pdram.tile([KVL + RP, BK], BF16)
        ag_out = pdram.tile([NC, KVL + RP, BK], BF16)
        aa_in = [pdram.tile([NC, QD, SSH], BF16, name=f"aain{i}")
                 for i in range(HPC)]
        aa_out = [pdram.tile([NC, QD, SSH], BF16, name=f"aaout{i}")
                  for i in range(HPC)]

        # ---- small constants ----
        ones_c = pc.tile([128, 1], BF16)
        nc.vector.memset(ones_c, 1.0)
        ones_r = pc.tile([1, 128], BF16)
        nc.vector.memset(ones_r, 1.0)
        ones8 = pc.tile([128, 2, 64], FP8)
        nc.vector.memset(ones8, 1.0)
        ebias = pc.tile([128, 1], F32)
        nc.vector.memset(ebias, -1.2)
        eps1 = pc.tile([1, 1], F32)
        nc.vector.memset(eps1, EPS_S)
        warm = pc.tile([1, 2], F32, name="actwarm")
        nc.scalar.activation(warm[0:1, 0:1], eps1[:], Sqrt)
        nc.scalar.activation(warm[0:1, 1:2], eps1[:], Exp)
        rotq = pc.tile([128, 128], BF16)
        nc.sync.dma_start(out=rotq, in_=d["rotq"])
        cosq = pc.tile([128, SSH], BF16)
        nc.sync.dma_start(out=cosq, in_=d["cosq"])
        sinq = pc.tile([128, SSH], BF16)
        nc.sync.dma_start(out=sinq, in_=d["sinq"])
        # global (stage A+F) tensors
        wkva_h = pg.tile([128, KT, KVL + RP], FP8)
        wkva_l = pg.tile([128, KT, KVL + RP], FP8)

        # =============== stage A: q path + back-shard kv_a ===============
        with tc.tile_pool(name="pwA", bufs=1) as pw, \
             tc.tile_pool(name="pA", bufs=1) as pa, \
             tc.tile_pool(name="pAs", bufs=2) as pas, \
             tc.tile_pool(name="ppA", bufs=3, space="PSUM") as ppa, \
             tc.tile_pool(name="ppSt", bufs=2, space="PSUM") as ppst, \
             tc.tile_pool(name="ppM", bufs=2, space="PSUM") as ppm:
            xsh_h = pw.tile([128, KT, SSH], FP8)
            xsh_l = pw.tile([128, KT, SSH], FP8)
            # back-shard x: slice of xf at columns FRONT..FRONT+BK
            xb_h = pw.tile([128, KT, BK], FP8)
            xb_l = pw.tile([128, KT, BK], FP8)
            # ordered load sequence on the SP queue: the DMA-engine pool is an
            # exclusive FIFO, so issue order IS service order. Back-shard
            # inputs first (gates the AllGather), then the q-path weights in
            # consumption-order column chunks so compute streams behind them.
            nc.sync.dma_start(out=xb_h, in_=d["xf_h"][:, :, FRONT:])
            nc.sync.dma_start(out=xb_l, in_=d["xf_l"][:, :, FRONT:])
            nc.sync.dma_start(out=wkva_h, in_=d["wkva_h"])
            nc.sync.dma_start(out=wkva_l, in_=d["wkva_l"])
            wqa_h = pw.tile([128, KT, QL], FP8)
            wqa_l = pw.tile([128, KT, QL], FP8)
            wqb_h = pw.tile([128, QLT, H * QD], FP8)
            wqb_l = pw.tile([128, QLT, H * QD], FP8)

            # --- back-shard kv LoRA-A (3-term DR) + rms ---
            bkvu = pa.tile([128, CT, BK], BF16, name="bkvu")
            sqb = pa.tile([128, CT, BK], BF16, name="sqb")
            kpb = pa.tile([RP, BK], BF16, name="kpb")
            p_bst = ppst.tile([1, BK], F32, tag="st", name="bstat")
            for m in range(CT + 1):
                mw = 128 if m < CT else RP
                p_a = ppa.tile([128, BK], F32, tag="a")
                _mm3(nc, p_a[:mw, :],
                     lambda p: wkva_h[:, 2 * p:2 * p + 2, m * 128:m * 128 + mw],
                     lambda p: wkva_l[:, 2 * p:2 * p + 2, m * 128:m * 128 + mw],
                     lambda p: xb_h[:, 2 * p:2 * p + 2, :],
                     lambda p: xb_l[:, 2 * p:2 * p + 2, :],
                     KT // 2, True, True)
                if m < CT:
                    nc.vector.tensor_copy(bkvu[:, m, :], p_a[:])
                    nc.vector.tensor_tensor(sqb[:, m, :], p_a[:], bkvu[:, m, :],
                                            Mul)
                else:
                    # raw kpe for own back shard -> staging rows KVL..
                    nc.vector.tensor_copy(kpb[:], p_a[:mw, :])
            # q-path loads gated on the first back-shard output so their FIFO
            # slots come after the back-shard inputs but alongside the rms
            # chain; the collective staging only queues ~6us behind them.
            gates = pc.tile([1, 32], BF16, name="gates")
            gate_state = {"tok": bkvu[0:1, 0, 0:1], "i": 0}

            def gated_dma(dst_tile, region, src):
                nc.sync.dma_start(out=dst_tile, in_=src)

            gated_dma(xsh_h, xsh_h[0:1, 0, 0:1], d["xsh_h"])
            gated_dma(xsh_l, xsh_l[0:1, 0, 0:1], d["xsh_l"])
            for m in range(CT):
                nc.tensor.matmul(p_bst[:], ones_c[:], sqb[:, m, :],
                                 start=(m == 0), stop=(m == CT - 1))
            rms_b = pas.tile([1, BK], BF16, tag="r1")
            nc.scalar.activation(rms_b[:], p_bst[:], Sqrt, scale=1.0 / KVL,
                                 bias=eps1[:])
            p_bb = ppm.tile([128, BK], F32, tag="m")
            nc.tensor.matmul(p_bb[:], ones_r[:], rms_b[:], start=True, stop=True)
            invb = pas.tile([128, BK], BF16, tag="r2")
            nc.vector.reciprocal(invb[:], p_bb[:])
            ckvb = pa.tile([128, CT, BK], BF16, name="ckvb")
            for m in range(CT):
                nc.vector.tensor_tensor(ckvb[:, m, :], bkvu[:, m, :], invb[:], Mul)
            nc.gpsimd.dma_start(
                out=ag_in[0:KVL, :].rearrange("(t p) c -> p t c", p=128),
                in_=ckvb[:])
            nc.gpsimd.dma_start(out=ag_in[KVL:, :], in_=kpb[:])
            # --- collective #1: AllGather back-shard latent+kpe ---
            nc.gpsimd.collective_compute(
                "AllGather", mybir.AluOpType.bypass, replica_groups=groups,
                ins=[ag_in[:].opt()], outs=[ag_out[:].opt()])
            # The tile scheduler orders by data deps, so dep-free bulk loads
            # would enter the shared DMA-engine FIFO ahead of the collective
            # staging. Gate each bulk chunk behind the previous one via a
            # write-after-read chain rooted at agmark (a tiny read of ag_in
            # that lands right after the staging writes).
            # agmark lands right after the AllGather staging is written; an
            # Activation-engine gate (activation reading the DMA's dest cell
            # with agmark as bias) makes each bulk load's FIFO slot follow
            # the staging without polluting the DVE SEQ.
            def act_gated_dma(dst_tile, region, src):
                # Pool-SEQ-timed gate: the gpsimd sequencer is blocked on the
                # AllGather's staging sems until the staging transfers land,
                # so this tiny read of the DMA's dest cell (WAR) executes at
                # staging-complete and admits the bulk load to the DMA FIFO
                # only after the collective staging — no token needed.
                g = gates[0:1, gate_state["i"] % 32:gate_state["i"] % 32 + 1]
                nc.gpsimd.tensor_tensor(g, region, ones_r[0:1, 0:1], Mul)
                nc.sync.dma_start(out=dst_tile, in_=src)
                gate_state["i"] += 1

            gate_state["fn"] = act_gated_dma
            for cc in range(3):
                cs = slice(cc * 512, (cc + 1) * 512)
                act_gated_dma(wqa_h[:, :, cs],
                              wqa_h[0:1, 0, cc * 512:cc * 512 + 1],
                              d["wqa_h"][:, :, cs])
                act_gated_dma(wqa_l[:, :, cs],
                              wqa_l[0:1, 0, cc * 512:cc * 512 + 1],
                              d["wqa_l"][:, :, cs])
            for cc in range(3):
                cs = slice(cc * 1024, (cc + 1) * 1024)
                act_gated_dma(wqb_h[:, :, cs],
                              wqb_h[0:1, 0, cc * 1024:cc * 1024 + 1],
                              d["wqb_h"][:, :, cs])
                act_gated_dma(wqb_l[:, :, cs],
                              wqb_l[0:1, 0, cc * 1024:cc * 1024 + 1],
                              d["wqb_l"][:, :, cs])

            # --- q LoRA-A (3-term DR) ---
            qlu_h = pa.tile([128, QLT, SSH], FP8, name="qluh")
            qlu_l = pa.tile([128, QLT, SSH], FP8, name="qlul")
            p_qst = ppst.tile([1, SSH], F32, tag="st", name="qstat")
            sqq = pa.tile([128, QLT, SSH], BF16, name="sqq")
            for k in range(QLT):
                p_a = ppa.tile([128, SSH], F32, tag="a")
                _mm3(nc, p_a[:],
                     lambda p: wqa_h[:, 2 * p:2 * p + 2, k * 128:(k + 1) * 128],
                     lambda p: wqa_l[:, 2 * p:2 * p + 2, k * 128:(k + 1) * 128],
                     lambda p: xsh_h[:, 2 * p:2 * p + 2, :],
                     lambda p: xsh_l[:, 2 * p:2 * p + 2, :],
                     KT // 2, True, True)
                nc.scalar.activation(qlu_h[:, k, :], p_a[:], Copy)
                nc.vector.tensor_tensor(qlu_l[:, k, :], p_a[:], qlu_h[:, k, :], Sub)
                nc.vector.tensor_tensor(sqq[:, k, :], p_a[:], qlu_h[:, k, :], Mul)
            for k in range(QLT):
                nc.tensor.matmul(p_qst[:], ones_c[:], sqq[:, k, :],
                                 start=(k == 0), stop=(k == QLT - 1))
            rms_q = pas.tile([1, SSH], BF16, tag="r1")
            nc.scalar.activation(rms_q[:], p_qst[:], Sqrt, scale=1.0 / QL,
                                 bias=eps1[:])
            p_bq = ppm.tile([128, SSH], F32, tag="m")
            nc.tensor.matmul(p_bq[:], ones_r[:], rms_q[:], start=True, stop=True)
            invq = pas.tile([128, SSH], F32, tag="r2", name="invq")
            nc.vector.reciprocal(invq[:], p_bq[:])

            # --- q_b (3-term DR): rope tiles first, then parity order ---
            q16 = pa.tile([128, H + NC, SSH], BF16, name="q16")

            def qb_group(mt):
                p_q = ppa.tile([128, SSH], F32, tag="a")
                _mm3(nc, p_q[:],
                     lambda p: wqb_h[:, 2 * p:2 * p + 2, mt * 128:(mt + 1) * 128],
                     lambda p: wqb_l[:, 2 * p:2 * p + 2, mt * 128:(mt + 1) * 128],
                     lambda p: qlu_h[:, 2 * p:2 * p + 2, :],
                     lambda p: qlu_l[:, 2 * p:2 * p + 2, :],
                     QLT // 2, True, True)
                nc.vector.tensor_tensor(q16[:, mt, :], p_q[:], invq[:], Mul)

            for mt in range(NC):
                qb_group(mt)
            for dd in range(NC):
                p_rq = ppm.tile([128, SSH], F32, tag="m")
                nc.tensor.matmul(p_rq[:], rotq[:], q16[:, dd, :],
                                 start=True, stop=True)
                rq16 = pas.tile([128, SSH], BF16, tag="rk", name="rq16")
                nc.vector.tensor_copy(rq16[:], p_rq[:])
                t1q = pas.tile([128, SSH], BF16, tag="t1")
                nc.vector.tensor_tensor(t1q[:], q16[:, dd, :], cosq[:], Mul)
                t2q = pas.tile([128, SSH], BF16, tag="t2")
                nc.vector.tensor_tensor(t2q[:], rq16[:], sinq[:], Mul)
                nc.vector.tensor_tensor(q16[:, dd, :], t1q[:], t2q[:],
                                        mybir.AluOpType.add)
            for mt in range(NC, NC + 8):
                qb_group(mt)
            nc.scalar.dma_start(
                out=aa_in[0][:, 0:NOPE, :].rearrange("j p c -> p j c"),
                in_=q16[:, 8:16, :])
            nc.scalar.dma_start(
                out=aa_in[0][:, NOPE:QD, :].rearrange("j p c -> p j c"),
                in_=q16[0:RP, 0:NC, :])
            nc.gpsimd.collective_compute(
                "AllToAll", mybir.AluOpType.bypass, replica_groups=groups,
                ins=[aa_in[0][:].opt()], outs=[aa_out[0][:].opt()])
            for mt in range(NC + 8, NC + 16):
                qb_group(mt)
            nc.scalar.dma_start(
                out=aa_in[1][:, 0:NOPE, :].rearrange("j p c -> p j c"),
                in_=q16[:, 16:24, :])
            nc.scalar.dma_start(
                out=aa_in[1][:, NOPE:QD, :].rearrange("j p c -> p j c"),
                in_=q16[RP:128, 0:NC, :])
            nc.gpsimd.collective_compute(
                "AllToAll", mybir.AluOpType.bypass, replica_groups=groups,
                ins=[aa_in[1][:].opt()], outs=[aa_out[1][:].opt()])

        # =============== stage F: front kv_a (replicated, 3-term DR) =========
        with tc.tile_pool(name="pg2", bufs=1) as pg2:
            ckv = pg2.tile([128, CT, S], BF16)          # assembled latent
            kpr = pg2.tile([RP, S], BF16, name="kpraw")  # raw kpe (x32)
            kv = {
                "wk": pg2.tile([128, CT, HPC * NOPE], BF16, name="wk"),
                "wv": pg2.tile([128, CT, HPC * VD], BF16, name="wv"),
                "kn": [pg2.tile([128, S], BF16, name=f"kn{h}")
                       for h in range(HPC)],
                "vst_h": pg2.tile([128, S // 128, HPC * VD], FP8, name="vsth"),
                "vst_l": pg2.tile([128, S // 128, HPC * VD], FP8, name="vstl"),
            }
            _stage_f(tc, d, ckv, kpr, wkva_h, wkva_l, ones_c, ones_r, eps1,
                     gates, gate_state, kv)
            _stage_b(tc, d, d_out, ckv, kpr, ag_out, aa_in, aa_out, ones_r,
                     ones8, ebias, rotq, gates, gate_state, pg2, kv)


def _kvb_cb(nc, ckv, kv, cb):
    """kv_b expansion for one 512-key block: kn both heads + v hi/lo."""
    tc_pool = kv["ppc"]
    Copy = mybir.ActivationFunctionType.Copy
    Sub = mybir.AluOpType.subtract
    cs = slice(cb * 512, (cb + 1) * 512)
    for h in range(HPC):
        p_k = tc_pool.tile([128, 512], mybir.dt.float32, tag="c")
        for t in range(CT):
            nc.tensor.matmul(p_k[:], kv["wk"][:, t, h * NOPE:(h + 1) * NOPE],
                             ckv[:, t, cs], start=(t == 0), stop=(t == CT - 1))
        nc.any.tensor_copy(kv["kn"][h][:, cs], p_k[:])
    for sb in range(cb * 4, cb * 4 + 4):
        p_v = tc_pool.tile([128, HPC * VD], mybir.dt.float32, tag="c")
        for t in range(CT):
            nc.tensor.matmul(p_v[:], ckv[:, t, sb * 128:(sb + 1) * 128],
                             kv["wv"][:, t, :], start=(t == 0),
                             stop=(t == CT - 1))
        nc.any.tensor_copy(kv["vst_h"][:, sb, :], p_v[:])
        nc.vector.tensor_tensor(kv["vst_l"][:, sb, :], p_v[:],
                                kv["vst_h"][:, sb, :], Sub)


def _stage_f(tc, d, ckv, kpr, wkva_h, wkva_l, ones_c, ones_r, eps1, gates,
             gate_state, kv):
    nc = tc.nc
    Sqrt = mybir.ActivationFunctionType.Sqrt
    Mul = mybir.AluOpType.mult

    if True:
        with tc.tile_pool(name="pwF", bufs=1) as pwf, \
             tc.tile_pool(name="pF", bufs=1) as pf, \
             tc.tile_pool(name="pFs", bufs=4) as pfs, \
             tc.tile_pool(name="ppF", bufs=3, space="PSUM") as ppf, \
             tc.tile_pool(name="ppFSt", bufs=2, space="PSUM") as ppfst, \
             tc.tile_pool(name="ppFM", bufs=2, space="PSUM") as ppfm:
            xf_h = pwf.tile([128, KT, FRONT], FP8)
            xf_l = pwf.tile([128, KT, FRONT], FP8)
            def gated_dma(dst_tile, region, src):
                gate_state["fn"](dst_tile, region, src)

            for q3 in range(FRONT // 512):
                cs = slice(q3 * 512, (q3 + 1) * 512)
                gated_dma(xf_h[:, :, cs], xf_h[0:1, 0, q3 * 512:q3 * 512 + 1],
                          d["xf_h"][:, :, cs])
                gated_dma(xf_l[:, :, cs], xf_l[0:1, 0, q3 * 512:q3 * 512 + 1],
                          d["xf_l"][:, :, cs])
            NCB = FRONT // 512                     # 3 col blocks
            fkvu = pf.tile([128, CT, FRONT], BF16, name="fkvu")
            sqf = pf.tile([128, CT, FRONT], BF16, name="sqf")
            for cb in range(NCB):
                cs = slice(cb * 512, (cb + 1) * 512)
                for m in range(CT + 1):
                    mw = 128 if m < CT else RP
                    p_f = ppf.tile([128, 512], F32, tag="f")
                    _mm3(nc, p_f[:mw, :],
                         lambda p: wkva_h[:, 2 * p:2 * p + 2, m * 128:m * 128 + mw],
                         lambda p: wkva_l[:, 2 * p:2 * p + 2, m * 128:m * 128 + mw],
                         lambda p: xf_h[:, 2 * p:2 * p + 2, cs],
                         lambda p: xf_l[:, 2 * p:2 * p + 2, cs],
                         KT // 2, True, True)
                    if m < CT:
                        nc.any.tensor_copy(fkvu[:, m, cs], p_f[:])
                        nc.any.tensor_tensor(sqf[:, m, cs], p_f[:],
                                             fkvu[:, m, cs], Mul)
                    else:
                        nc.any.tensor_copy(kpr[:, cb * 512:(cb + 1) * 512],
                                           p_f[:mw, :])
                p_fst = ppfst.tile([1, 512], F32, tag="fst")
                for m in range(CT):
                    nc.tensor.matmul(p_fst[:], ones_c[:], sqf[:, m, cs],
                                     start=(m == 0), stop=(m == CT - 1))
                rms_f = pfs.tile([1, 512], BF16, tag="fr1")
                nc.scalar.activation(rms_f[:], p_fst[:], Sqrt, scale=1.0 / KVL,
                                     bias=eps1[:])
                p_fb = ppfm.tile([128, 512], F32, tag="fm")
                nc.tensor.matmul(p_fb[:], ones_r[:], rms_f[:], start=True,
                                 stop=True)
                invf = pfs.tile([128, 512], BF16, tag="fr2")
                nc.vector.reciprocal(invf[:], p_fb[:])
                for m in range(CT):
                    nc.any.tensor_tensor(ckv[:, m, cs], fkvu[:, m, cs],
                                         invf[:], Mul)

def _stage_b(tc, d, d_out, ckv, kpr, ag_out, aa_in, aa_out, ones_r, ones8,
             ebias, rotq, gates, gate_state, pg2, kv):
    nc = tc.nc
    Exp = mybir.ActivationFunctionType.Exp
    Copy = mybir.ActivationFunctionType.Copy
    Mul = mybir.AluOpType.mult
    Sub = mybir.AluOpType.subtract

    if True:
        # =============== stage B: assemble + head-local attention ============
        with tc.tile_pool(name="pB", bufs=1) as pb, \
             tc.tile_pool(name="pBe", bufs=13) as pbe, \
             tc.tile_pool(name="pBo", bufs=4) as pbo, \
             tc.tile_pool(name="pBn", bufs=4) as pbn, \
             tc.tile_pool(name="ppS", bufs=4, space="PSUM") as pps, \
             tc.tile_pool(name="ppO", bufs=1, space="PSUM") as ppo, \
             tc.tile_pool(name="ppD", bufs=1, space="PSUM") as ppd, \
             tc.tile_pool(name="ppC", bufs=2, space="PSUM") as ppc:
            kn, vst_h, vst_l = kv["kn"], kv["vst_h"], kv["vst_l"]
            wo_h = pg2.tile([128, HPC, HID], FP8)
            wo_l = pg2.tile([128, HPC, HID], FP8)
            msk = pg2.tile([128, 4, SB], F32)
            cosk = pg2.tile([RP, S], BF16)
            sink = pg2.tile([RP, S], BF16)
            def gated_dma(dst_tile, region, src):
                gate_state["fn"](dst_tile, region, src)

            gated_dma(kv["wk"], kv["wk"][0:1, 0, 0:1], d["wk"])
            gated_dma(kv["wv"], kv["wv"][0:1, 0, 0:1], d["wv"])
            gated_dma(cosk, cosk[0:1, 0:1], d["cosk"])
            gated_dma(sink, sink[0:1, 0:1], d["sink"])
            gated_dma(wo_h, wo_h[0:1, 0, 0:1], d["wo_h"])
            gated_dma(wo_l, wo_l[0:1, 0, 0:1], d["wo_l"])
            gated_dma(msk, msk[0:1, 0, 0:1], d["msk"])
            # unpack AG: back latent + kpe into global tiles
            for t in range(CT):
                nc.gpsimd.dma_start(
                    out=ckv[:, t, FRONT:].rearrange("p (j c) -> p j c", j=NC),
                    in_=ag_out[:, t * 128:(t + 1) * 128, :].rearrange(
                        "j p c -> p j c"))
            nc.gpsimd.dma_start(
                out=kpr[:, FRONT:].rearrange("p (j c) -> p j c", j=NC),
                in_=ag_out[:, KVL:, :].rearrange("j p c -> p j c"))
            # kpe rope over full seq (cosk/sink carry the 1/SW fold)
            kpdg = pg2.tile([RP, S], BF16)
            for cb in range(S // 512):
                cs = slice(cb * 512, (cb + 1) * 512)
                p_rk = ppc.tile([128, 512], F32, tag="c", name="rotk")
                nc.tensor.matmul(p_rk[:RP, :], rotq[0:RP, 0:RP], kpr[:, cs],
                                 start=True, stop=True)
                rk16 = pbn.tile([RP, 512], BF16, tag="rk")
                nc.vector.tensor_copy(rk16[:], p_rk[:RP, :])
                t1 = pbn.tile([RP, 512], BF16, tag="t1")
                nc.vector.tensor_tensor(t1[:], kpr[:, cs], cosk[:, cs], Mul)
                t2 = pbn.tile([RP, 512], BF16, tag="t2")
                nc.vector.tensor_tensor(t2[:], rk16[:], sink[:, cs], Mul)
                nc.vector.tensor_tensor(kpdg[:, cs], t1[:], t2[:],
                                        mybir.AluOpType.add)

            qt = [pg2.tile([128, S], BF16, name=f"qt{h}") for h in range(HPC)]
            qpt = [pg2.tile([RP, S], BF16, name=f"qpt{h}") for h in range(HPC)]

            def unpack_q(h):
                nc.gpsimd.dma_start(
                    out=qt[h][:].rearrange("p (j c) -> p j c", j=NC),
                    in_=aa_out[h][:, 0:NOPE, :].rearrange("j p c -> p j c"))
                nc.gpsimd.dma_start(
                    out=qpt[h][:].rearrange("p (j c) -> p j c", j=NC),
                    in_=aa_out[h][:, NOPE:QD, :].rearrange("j p c -> p j c"))

            # --- kv_b for the gathered back block (front blocks were
            # interleaved into stage F) ---
            kv["ppc"] = ppc
            for cb in range(S // 512):
                _kvb_cb(nc, ckv, kv, cb)

            # --- attention: heads outer, software-pipelined (as v2) ---
            ao_h = pg2.tile([128, NSB, HPC, SB], FP8, name="aoh")
            ao_l = pg2.tile([128, NSB, HPC, SB], FP8, name="aol")
            pending = None

            def finisher(fin):
                h, qb, p_o, p_d = fin
                den = pbn.tile([1, SB], BF16, tag="den")
                nc.vector.tensor_copy(den[:], p_d[0:1, :])
                p_b = ppc.tile([128, SB], F32, tag="c", name="bcast")
                nc.tensor.matmul(p_b[:], ones_r[:], den[:], start=True, stop=True)
                rec = pbn.tile([128, SB], F32, tag="rec")
                nc.vector.reciprocal(rec[:], p_b[:])
                aot = pbn.tile([128, SB], BF16, tag="aot")
                nc.vector.tensor_tensor(aot[:], p_o[:], rec[:], Mul)
                nc.any.tensor_copy(ao_h[:, qb, h, :], aot[:])
                nc.any.tensor_tensor(ao_l[:, qb, h, :], aot[:],
                                     ao_h[:, qb, h, :], Sub)

            def oproj(qb):
                for st in range(SB // 128):
                    sc = slice(qb * SB + st * 128, qb * SB + (st + 1) * 128)
                    ot = pbo.tile([128, HID], BF16, tag="ot")
                    for nb in range(HID // SB):
                        ncols = bass.ts(nb, SB)
                        p_c = ppc.tile([128, SB], F32, tag="c")
                        aoh_ = ao_h[:, qb, :, st * 128:(st + 1) * 128]
                        aol_ = ao_l[:, qb, :, st * 128:(st + 1) * 128]
                        nc.tensor.matmul(p_c[:], aoh_, wo_h[:, :, ncols],
                                         start=True, stop=False, perf_mode=DR)
                        nc.tensor.matmul(p_c[:], aoh_, wo_l[:, :, ncols],
                                         start=False, stop=False, perf_mode=DR)
                        nc.tensor.matmul(p_c[:], aol_, wo_h[:, :, ncols],
                                         start=False, stop=True, perf_mode=DR)
                        nc.vector.tensor_scalar_mul(ot[:, ncols], p_c[:],
                                                    1.0 / SW)
                    nc.sync.dma_start(out=d_out[sc, :], in_=ot[:])

            for h in range(HPC):
                unpack_q(h)
                for qb in range(NSB):
                    qcols = bass.ts(qb, SB)
                    nk = 4 * (qb + 1)
                    npair = nk // 2
                    p_o = ppo.tile([128, SB], F32, tag="o")
                    p_d = ppd.tile([64, SB], F32, tag="d")
                    ework = []

                    def av_den(pp, e2_):
                        vs = slice(2 * pp, 2 * pp + 2)
                        hv = slice(h * VD, (h + 1) * VD)
                        nc.tensor.matmul(p_o[:], vst_h[:, vs, hv], e2_[:],
                                         start=(pp == 0), stop=False,
                                         perf_mode=DR)
                        nc.tensor.matmul(p_o[:], vst_l[:, vs, hv], e2_[:],
                                         start=False, stop=(pp == npair - 1),
                                         perf_mode=DR)
                        nc.tensor.matmul(p_d[:], ones8[:], e2_[:],
                                         start=(pp == 0), stop=(pp == npair - 1),
                                         perf_mode=DR)

                    for pp in range(npair):
                        e2 = pbe.tile([128, 2, SB], FP8, tag="e")
                        for j in range(2):
                            ik = 2 * pp + j
                            kc = slice(ik * 128, (ik + 1) * 128)
                            p_s = pps.tile([128, SB], F32, tag="s")
                            nc.tensor.matmul(p_s[:], kn[h][:, kc], qt[h][:, qcols],
                                             start=True, stop=False)
                            nc.tensor.matmul(p_s[:], kpdg[:, kc], qpt[h][:, qcols],
                                             start=False, stop=True)
                            if ik == 3 and pending is not None:
                                fin, oqb = pending
                                finisher(fin)
                                pending = None
                                if oqb is not None:
                                    oproj(oqb)
                            r = ik - 4 * qb
                            if r >= 0:
                                nc.vector.tensor_tensor(p_s[:], p_s[:],
                                                        msk[:, r, :],
                                                        mybir.AluOpType.add)
                            nc.scalar.activation(e2[:, j, :], p_s[:], Exp,
                                                 scale=SCALE / SW, bias=ebias[:])
                        ework.append((pp, e2))
                        if len(ework) == 3:
                            av_den(*ework.pop(0))
                    for item in ework:
                        av_den(*item)
                    pending = ((h, qb, p_o, p_d),
                               qb if h == HPC - 1 else None)
            fin, oqb = pending
            finisher(fin)
            if oqb is not None:
                oproj(oqb)


def _host_constants():
    inv_freq = 1.0 / (ROPE_THETA ** (np.arange(0, RP, 2, dtype=np.float32) / RP))
    t = np.arange(S, dtype=np.float32)
    freqs = np.outer(t, inv_freq)
    emb = np.concatenate([freqs, freqs], -1)          # [S, 64]
    cos, sin = np.cos(emb), np.sin(emb)
    cosq = np.concatenate([cos.T, cos.T], 0).astype(np.float32)   # [128, S]
    sinq = np.concatenate([sin.T, sin.T], 0).astype(np.float32)
    cosk = (cos.T / SW).astype(np.float32)            # [64, S], 1/SW folded
    sink = (sin.T / SW).astype(np.float32)

    mska = np.zeros((128, 4, SB), np.float32)
    for r in range(4):
        for p in range(128):
            mska[p, r, :p + 128 * r] = NEG
    Q = np.zeros((RP, RP), np.float32)
    for i in range(RP // 2):
        Q[i, i + RP // 2] = -1.0
        Q[i + RP // 2, i] = 1.0
    P = np.zeros((128, 128), np.float32)
    P[:RP, :RP] = Q
    P[RP:, RP:] = Q
    rotq = P.T.copy()
    return cosq, sinq, cosk, sink, mska, rotq


def _tile3(w, kt):
    """[kt*128, F] -> [128, kt, F]"""
    return np.ascontiguousarray(
        w.reshape(kt, 128, w.shape[1]).transpose(1, 0, 2))


def _split8(w):
    """scaled hi/lo fp8 split (already-scaled input)."""
    hi = w.astype(NPF8)
    lo = (w - hi.astype(np.float32)).astype(NPF8)
    return hi, lo


def kernel(hidden_states, w_q_a, q_a_weight, w_q_b, w_kv_a, kv_a_weight,
           w_kv_b, w_o):
    global LAST_RESULT
    if "nc" not in _CACHE:
        _CACHE["nc"] = _build_program()
    nc = _CACHE["nc"]

    x = np.asarray(hidden_states, np.float32)[0]       # [S, 2048]
    xt = np.ascontiguousarray(x.T)                     # [2048, S]
    wqa_t = np.asarray(w_q_a, np.float32).T * SW       # [HID, QL] x32
    wkva_t = np.asarray(w_kv_a, np.float32).T * SW     # [HID, 576] x32
    wqb_eff = np.asarray(w_q_b, np.float32) * np.asarray(q_a_weight, np.float32)[None, :]
    wkvb_eff = np.asarray(w_kv_b, np.float32) * np.asarray(kv_a_weight, np.float32)[None, :]
    won = np.asarray(w_o, np.float32)                  # [HID, H*VD]

    # q_b output feature permutation: cols [0:1024] rope packed 2-heads/tile,
    # [1024:2048] even heads' nope, [2048:3072] odd heads' nope — matching the
    # device-side consumption order so wqb column-chunk loads stream.
    perm = np.zeros(H * QD, np.int64)
    for dd in range(NC):
        for j in range(HPC):
            hh = 2 * dd + j
            perm[dd * 128 + j * RP: dd * 128 + (j + 1) * RP] = \
                hh * QD + NOPE + np.arange(RP)
    for ei in range(8):
        perm[1024 + ei * 128: 1024 + (ei + 1) * 128] = (2 * ei) * QD + np.arange(NOPE)
    for oi in range(8):
        perm[2048 + oi * 128: 2048 + (oi + 1) * 128] = (2 * oi + 1) * QD + np.arange(NOPE)
    wqb_p = np.ascontiguousarray(wqb_eff[perm, :].T) * SW   # [QL, 3072] x32

    cosq, sinq, cosk, sink, mska, rotq = _host_constants()

    xt_h, xt_l = _split8(xt)                           # full [2048, S]
    wqa_h, wqa_l = _split8(wqa_t)
    wkva_h, wkva_l = _split8(wkva_t)
    wqb_h, wqb_l = _split8(wqb_p)

    shared = {
        "wqa_h": _tile3(wqa_h, KT), "wqa_l": _tile3(wqa_l, KT),
        "wkva_h": _tile3(wkva_h, KT), "wkva_l": _tile3(wkva_l, KT),
        "wqb_h": _tile3(wqb_h, QLT), "wqb_l": _tile3(wqb_l, QLT),
        "maskadd": mska, "rotq16": rotq.astype(NPBF),
        "cosk": cosk.astype(NPBF), "sink": sink.astype(NPBF),
    }

    in_maps = []
    for c in range(NC):
        h0, h1 = HPC * c, HPC * c + 1
        wk_t = np.concatenate(
            [wkvb_eff[h * (NOPE + VD):h * (NOPE + VD) + NOPE] for h in (h0, h1)],
            0).T
        wv_t = np.concatenate(
            [wkvb_eff[h * (NOPE + VD) + NOPE:(h + 1) * (NOPE + VD)] for h in (h0, h1)],
            0).T
        wo_t = np.stack(
            [np.ascontiguousarray(won[:, h * VD:(h + 1) * VD].T) for h in (h0, h1)],
            1) * SW                                     # [128, 2, HID] x32
        wo_hc, wo_lc = _split8(wo_t)
        cols = slice(c * SSH, (c + 1) * SSH)
        bcols = np.r_[0:FRONT, FRONT + c * BK:FRONT + (c + 1) * BK]
        im = dict(shared)
        im.update({
            "xsh_h": _tile3(np.ascontiguousarray(xt_h[:, cols].astype(np.float32)), KT).astype(NPF8),
            "xsh_l": _tile3(np.ascontiguousarray(xt_l[:, cols].astype(np.float32)), KT).astype(NPF8),
            "xf_h": _tile3(np.ascontiguousarray(xt_h[:, bcols].astype(np.float32)), KT).astype(NPF8),
            "xf_l": _tile3(np.ascontiguousarray(xt_l[:, bcols].astype(np.float32)), KT).astype(NPF8),
            "wk16": _tile3(wk_t, CT).astype(NPBF),
            "wv16": _tile3(wv_t, CT).astype(NPBF),
            "wo_h": np.ascontiguousarray(wo_hc),
            "wo_l": np.ascontiguousarray(wo_lc),
            "cosq": np.ascontiguousarray(cosq[:, cols]).astype(NPBF),
            "sinq": np.ascontiguousarray(sinq[:, cols]).astype(NPBF),
        })
        in_maps.append(im)

    res = run_bass_kernel_spmd(nc, in_maps, list(range(NC)))
    LAST_RESULT = res
    out = np.zeros((S, HID), np.float32)
    for c in range(NC):
        out += np.asarray(res.results[c]["out"]).astype(np.float32)
    return out.reshape(1, S, HID)
